# revision 38
# baseline (speedup 1.0000x reference)
"""Trainium2 Bass kernel: dense-masked sliding-window attention.

nn.Module semantics (see harness reference):
    B,S,E,H,W = 1, 4096, 1024, 16, 256; D = 64
    qkv = x @ w_qkv -> q,k,v  [B,S,H,D]
    scores = q k^T / sqrt(D), masked to the sliding causal window
             (key j allowed for query i iff i-W < j <= i)
    out = softmax(scores) v  -> [B,S,E] @ w_out

Sharding: sequence-parallel over 8 NeuronCores. Core c owns queries
[512c, 512c+512) and receives a 256-row key/value halo on the left; no
collectives are needed (host concatenates the per-core output rows).

Pipeline (all fp16 matmuls; PE-bound at ~89us of moving rows):
  - q^T with the contraction loop OUTER across all 8 PSUM banks so the
    PE streams behind the initial wq/x DMAs without stalling.
  - v computed next (needed by every attention pair).
  - per head-pair: k^T projection + QK + exp + mask + attn*V + normalize
    are interleaved so the activation-engine exp (the attention-phase
    cap) hides under the pair's PE work.
  - scores laid out block-per-128-cols with the 9 mask-needing blocks
    first (cols 0:1152) and the 3 always-inside-window blocks last, so
    masking is one DVE multiply over a 1152-wide fp16 span.
  - softmax skips max-subtraction (scores are O(1)); denominators fall
    out of attn*V via a ones column per head; reciprocal on DVE straight
    from PSUM; broadcast + normalize-multiply on the idle Pool engine.
"""

import numpy as np
from contextlib import ExitStack

import concourse.bass as bass
import concourse.tile as tile
from concourse import bacc, mybir
from concourse.bass_utils import run_bass_kernel_spmd

F32 = mybir.dt.float32
F16 = mybir.dt.float16

B, S, E, H, W = 1, 4096, 1024, 16, 256
D = E // H  # 64
SCALE = D ** -0.5
N_CORES = 8
SQ = S // N_CORES          # 512 queries per core
HALO = W                   # 256 halo keys
SK = SQ + HALO             # 768 key rows per core
KC = E // 128              # 8 contraction chunks
QB = SQ // 128             # 4 query blocks per core
TC = SK // 128             # 6 key chunks per core
VW = H * (D + 1)           # 1040: v row width with ones columns

# Score-block layout, one 128-col slot per (T, qb) band block.
# Block types by qb-T+2: 0 = causal diagonal (mask), 1 = fully inside
# window (no mask except (1,0): halo garbage on core 0), 2 = window tail
# (mask).  Masked blocks occupy cols [0, 1152) grouped by T; free blocks
# cols [1152, 1536).
_MASKED = [(0, 0), (1, 0), (1, 1), (2, 0), (2, 2), (3, 1), (3, 3),
           (4, 2), (5, 3)]
_FREE = [(2, 1), (3, 2), (4, 3)]
BLOCKS = _MASKED + _FREE                     # (T, qb) -> col slot order
BCOL = {tq: 128 * i for i, tq in enumerate(BLOCKS)}
NMASK_COLS = 128 * len(_MASKED)              # 1152
NS_COLS = 128 * len(BLOCKS)                  # 1536
# attnv accumulation order per qb: the 3 T chunks of its band
QB_T = {qb: [qb, qb + 1, qb + 2] for qb in range(QB)}


def _emit_body(ctx: ExitStack, tc_: "tile.TileContext", xT_d, wq_d, wk_d, wv_d,
               wout_d, pmask_d, out_d):
    nc = tc_.nc
    P = 128

    xt_pool = ctx.enter_context(tc_.tile_pool(name="xt", bufs=KC))
    w_pool = ctx.enter_context(tc_.tile_pool(name="w", bufs=10))
    qt_pool = ctx.enter_context(tc_.tile_pool(name="qt", bufs=KC))
    kt_pool = ctx.enter_context(tc_.tile_pool(name="kt", bufs=2))
    v_pool = ctx.enter_context(tc_.tile_pool(name="v", bufs=TC))
    pm_pool = ctx.enter_context(tc_.tile_pool(name="pm", bufs=1))
    et_pool = ctx.enter_context(tc_.tile_pool(name="et", bufs=4))
    at_pool = ctx.enter_context(tc_.tile_pool(name="at", bufs=KC))
    os_pool = ctx.enter_context(tc_.tile_pool(name="os", bufs=4))
    nrm_pool = ctx.enter_context(tc_.tile_pool(name="nrm", bufs=4))
    # PSUM: poolBig 2x[128,1536] (6 banks) + poolSmall 2x[128,512] (2)
    ps_big = ctx.enter_context(tc_.tile_pool(name="psb", bufs=2, space="PSUM"))
    ps_sm = ctx.enter_context(tc_.tile_pool(name="pss", bufs=2, space="PSUM"))

    # ---- DMA issue order: (wq, xt) pairs gate the q^T k-loop; wv DMAs
    # slot into the q^T consumption slack so the v phase never stalls.
    wq, xt, wv = [], [], []
    for k in range(KC):
        t = w_pool.tile([P, 1024], F16, tag="w", name=f"wq{k}")
        wq.append(t)
        t = xt_pool.tile([P, SK], F16, tag="xt", name=f"xt{k}")
        xt.append(t)
        t = w_pool.tile([P, 1024], F16, tag="wv", name=f"wv{k}")
        wv.append(t)
    wv_at = {2: 0, 4: 1, 6: 2}  # after (wq,xt) pair k, issue wv chunk v
    for k in range(KC):
        nc.sync.dma_start(wq[k][:], wq_d.ap()[k * P:(k + 1) * P, :])
        nc.sync.dma_start(xt[k][:], xT_d.ap()[k * P:(k + 1) * P, :])
        if k in wv_at:
            v0 = wv_at[k]
            nc.sync.dma_start(wv[v0][:], wv_d.ap()[v0 * P:(v0 + 1) * P, :])
    for v0 in range(3, KC):
        nc.sync.dma_start(wv[v0][:], wv_d.ap()[v0 * P:(v0 + 1) * P, :])
    pm = pm_pool.tile([P, NMASK_COLS], F16)
    nc.sync.dma_start(pm[:], pmask_d.ap()[:])
    wk = []
    for k in range(KC):
        t = w_pool.tile([P, 1024], F16, tag="w", name=f"wk{k}")
        nc.sync.dma_start(t[:], wk_d.ap()[k * P:(k + 1) * P, :])
        wk.append(t)
    wo = []
    for k in range(KC):
        t = w_pool.tile([P, 1024], F16, tag="w", name=f"wo{k}")
        nc.sync.dma_start(t[:], wout_d.ap()[k * P:(k + 1) * P, :])
        wo.append(t)

    # ---- q^T [E, SQ]: contraction OUTER across all 8 PSUM banks ----------
    # big tiles hold n-chunks 0..5 (3 each), small tiles n=6,7.
    qb_ps = [ps_big.tile([P, 1536], F32, tag="psb", name=f"qtp{i}")
             for i in range(2)]
    qs_ps = [ps_sm.tile([P, 512], F32, tag="pss", name=f"qts{i}")
             for i in range(2)]

    def _q_dst(n):
        if n >= 2:
            m = n - 2
            return qb_ps[m // 3][:, (m % 3) * 512:(m % 3) * 512 + 512]
        return qs_ps[n][:, :]

    for k in range(KC - 1):
        for n in range(KC):
            nc.tensor.matmul(_q_dst(n), wq[k][:, n * P:(n + 1) * P],
                             xt[k][:, HALO:SK], start=(k == 0), stop=False)
    qt = []
    for n in range(KC):
        nc.tensor.matmul(_q_dst(n), wq[KC - 1][:, n * P:(n + 1) * P],
                         xt[KC - 1][:, HALO:SK], start=False, stop=True)
        t = qt_pool.tile([P, SQ], F16, tag="qt")
        nc.scalar.copy(t[:], _q_dst(n))
        qt.append(t)

    # ---- attention helpers (defined early; kproj(0) overlaps the v tail) -
    at = [at_pool.tile([P, SQ], F16, tag="at", name=f"at{i}")
          for i in range(KC)]
    ktt = [None] * KC
    sps = [None] * H
    ets = [None] * H
    ots = [None] * H
    vt = []

    def emit_kproj(p):
        kp = ps_big.tile([P, 1536], F32, tag="psb", name=f"ktp{p}")
        for k in range(KC):
            nc.tensor.matmul(kp[:, 0:512], wk[k][:, p * P:(p + 1) * P],
                             xt[k][:, 0:512], start=(k == 0), stop=(k == KC - 1))
            nc.tensor.matmul(kp[:, 512:768], wk[k][:, p * P:(p + 1) * P],
                             xt[k][:, 512:768], start=(k == 0),
                             stop=(k == KC - 1))
        ktp = kt_pool.tile([P, SK], F16, tag="kt")
        for c0 in (0, 256, 512):
            nc.vector.tensor_copy(ktp[:, c0:c0 + 256], kp[:, c0:c0 + 256])
        ktt[p] = ktp

    # ---- v natural [SK, 16*(64+1)] --------------------------------------
    ones_f = nrm_pool.tile([P, 1], F32, tag="ones")
    nc.vector.memset(ones_f[:], 1.0)
    for sc in range(TC):
        if sc == TC - 1:
            emit_kproj(0)
        t = v_pool.tile([P, VW], F16, tag="v")
        tv = t[:].rearrange("p (h c) -> p h c", h=H)
        if sc == 0:
            # first chunk uses the two small-bank tiles (freed earliest)
            halves = [ps_sm.tile([P, SQ], F32, tag="pss", name=f"v0h{i}")
                      for i in range(2)]
            for k in range(KC):
                for i in range(2):
                    nc.tensor.matmul(halves[i][:], xt[k][:, sc * P:(sc + 1) * P],
                                     wv[k][:, i * 512:(i + 1) * 512],
                                     start=(k == 0), stop=(k == KC - 1))
            for i in range(2):
                nc.vector.tensor_copy(
                    tv[:, i * 8:(i + 1) * 8, 0:D],
                    halves[i][:].rearrange("p (h c) -> p h c", h=8))
        else:
            ps = ps_big.tile([P, 1536], F32, tag="psb")
            for k in range(KC):
                nc.tensor.matmul(ps[:, 0:512], xt[k][:, sc * P:(sc + 1) * P],
                                 wv[k][:, 0:512], start=(k == 0),
                                 stop=(k == KC - 1))
                nc.tensor.matmul(ps[:, 512:1024], xt[k][:, sc * P:(sc + 1) * P],
                                 wv[k][:, 512:1024], start=(k == 0),
                                 stop=(k == KC - 1))
            nc.vector.tensor_copy(tv[:, :, 0:D],
                                  ps[:, 0:1024].rearrange("p (h c) -> p h c",
                                                          h=H))
        nc.vector.tensor_copy(tv[:, :, D:D + 1],
                              ones_f[:, None, :].broadcast_to([P, H, 1]))
        vt.append(t)

    # ---- attention: sub-level software pipeline, attnv lags QK by LAG ----
    # Per head pair p: k^T chunk projection for pair p+1 is PE filler
    # while pair p's exp/mask (Act/DVE) complete.
    def emit_qk(h):
        p, sub = h // 2, h % 2
        r0 = 64 * sub
        sp = ps_big.tile([P, 1536], F32, tag="psb", name=f"s{h}")
        sps[h] = sp
        for (T, qb) in BLOCKS:
            c0 = BCOL[(T, qb)]
            nc.tensor.matmul(
                sp[:, c0:c0 + 128],
                ktt[p][r0:r0 + 64, T * P:(T + 1) * P],
                qt[p][r0:r0 + 64, qb * 128:(qb + 1) * 128],
                start=True, stop=True, tile_position=(r0, 0))
        et = et_pool.tile([P, NS_COLS], F16, tag="et")
        ets[h] = et
        if h >= H - 4:
            # pipeline-drain subs: split exp so the lagging attnv can
            # start after the first half
            nc.scalar.activation(et[:, 0:768], sp[:, 0:768],
                                 mybir.ActivationFunctionType.Exp)
            nc.vector.tensor_tensor(et[:, 0:768], et[:, 0:768],
                                    pm[:, 0:768], mybir.AluOpType.mult)
            nc.scalar.activation(et[:, 768:NS_COLS], sp[:, 768:NS_COLS],
                                 mybir.ActivationFunctionType.Exp)
            nc.vector.tensor_tensor(et[:, 768:NMASK_COLS],
                                    et[:, 768:NMASK_COLS],
                                    pm[:, 768:NMASK_COLS],
                                    mybir.AluOpType.mult)
        else:
            nc.scalar.activation(et[:], sp[:], mybir.ActivationFunctionType.Exp)
            nc.vector.tensor_tensor(et[:, 0:NMASK_COLS], et[:, 0:NMASK_COLS],
                                    pm[:], mybir.AluOpType.mult)

    def emit_av(h):
        p, sub = h // 2, h % 2
        r0 = 64 * sub
        et = ets[h]
        ot = ps_sm.tile([P, SQ], F32, tag="pss", name=f"o{h}")
        ots[h] = ot
        for qb in range(QB):
            for j, T in enumerate(QB_T[qb]):
                c0 = BCOL[(T, qb)]
                nc.tensor.matmul(
                    ot[0:65, qb * 128:(qb + 1) * 128],
                    vt[T][:, h * 65:h * 65 + 65],
                    et[:, c0:c0 + 128],
                    start=(j == 0), stop=(j == 2))
        rc = nrm_pool.tile([1, SQ], F32, tag="rc")
        nc.vector.reciprocal(rc[:], ot[64:65, :])
        rb = nrm_pool.tile([64, SQ], F32, tag="rb")
        nc.gpsimd.partition_broadcast(rb[:], rc[:])
        nc.vector.tensor_tensor(at[p][r0:r0 + 64, :], ot[0:64, :], rb[:],
                                mybir.AluOpType.mult)

    # out projection, interleaved into the attention pipeline drain:
    # sb 0/1 start their first 6 contraction steps while the last attnv
    # slots retire.
    ops = [None] * QB

    def emit_out(sb, p0, p1):
        if ops[sb] is None:
            ops[sb] = ps_big.tile([P, 1536], F32, tag="psb", name=f"op{sb}")
        ps = ops[sb]
        for p in range(p0, p1):
            nc.tensor.matmul(ps[:, 0:512], at[p][:, sb * P:(sb + 1) * P],
                             wo[p][:, 0:512], start=(p == 0), stop=(p == KC - 1))
            nc.tensor.matmul(ps[:, 512:1024], at[p][:, sb * P:(sb + 1) * P],
                             wo[p][:, 512:1024], start=(p == 0),
                             stop=(p == KC - 1))

    def emit_out_store(sb, quarters=False):
        ps = ops[sb]
        n = 4 if quarters else 2
        w = 1024 // n
        for piece in range(n):
            ob = os_pool.tile([P, w], F32, tag="os", name=f"ob{sb}{piece}")
            nc.scalar.copy(ob[:], ps[:, piece * w:piece * w + w])
            nc.sync.dma_start(
                out_d.ap()[sb * P:(sb + 1) * P, piece * w:piece * w + w],
                ob[:])

    LAG = 3
    for h in range(H + LAG):
        if h < H:
            p, sub = h // 2, h % 2
            if sub == 1 and p + 1 < KC:
                emit_kproj(p + 1)
            emit_qk(h)
        if h >= LAG:
            emit_av(h - LAG)

    # ---- output projection ----------------------------------------------
    for sb in range(QB):
        emit_out(sb, 0, KC)
        emit_out_store(sb, quarters=(sb == QB - 1))


def build(n_iters: int = 1):
    nc = bacc.Bacc("TRN2", target_bir_lowering=False, debug=False,
                   num_devices=N_CORES)
    xT_d = nc.dram_tensor("xT", [E, SK], F16, kind="ExternalInput")
    wq_d = nc.dram_tensor("wq", [E, E], F16, kind="ExternalInput")
    wk_d = nc.dram_tensor("wk", [E, E], F16, kind="ExternalInput")
    wv_d = nc.dram_tensor("wv", [E, E], F16, kind="ExternalInput")
    wout_d = nc.dram_tensor("wout", [E, E], F16, kind="ExternalInput")
    pmask_d = nc.dram_tensor("pmask", [128, NMASK_COLS], F16,
                             kind="ExternalInput")
    out_d = nc.dram_tensor("out", [SQ, E], F32, kind="ExternalOutput")
    with tile.TileContext(nc) as tc_, ExitStack() as ctx:
        if n_iters > 1:
            with tc_.For_i(0, n_iters, 1):
                _emit_body(ctx, tc_, xT_d, wq_d, wk_d, wv_d, wout_d, pmask_d,
                           out_d)
        else:
            _emit_body(ctx, tc_, xT_d, wq_d, wk_d, wv_d, wout_d, pmask_d,
                       out_d)
    nc.compile()
    return nc


def make_in_maps(x, allowed_mask, w_qkv, w_out):
    """Shard the full inputs into per-core input maps (host marshaling)."""
    x2 = np.asarray(x, dtype=np.float32).reshape(S, E)
    wqkv = np.asarray(w_qkv, dtype=np.float32)
    wq = np.ascontiguousarray(wqkv[:, 0:E]) * np.float32(SCALE)
    wk = np.ascontiguousarray(wqkv[:, E:2 * E])
    wv = np.ascontiguousarray(wqkv[:, 2 * E:3 * E])
    wout = np.ascontiguousarray(np.asarray(w_out, dtype=np.float32))
    am = np.asarray(allowed_mask).reshape(S, S)

    xT = np.ascontiguousarray(x2.T)  # [E, S]
    in_maps = []
    for c in range(N_CORES):
        lo = c * SQ - HALO
        xTc = np.zeros((E, SK), dtype=np.float32)
        ofs = max(0, -lo)
        xTc[:, ofs:] = xT[:, lo + ofs:c * SQ + SQ]
        pmask = np.zeros((128, NMASK_COLS), dtype=np.float32)
        for (T, qb) in _MASKED:
            col0 = BCOL[(T, qb)]
            t0 = lo + T * 128
            if t0 + 128 <= 0:
                continue
            tlo = max(0, -t0)
            s0 = c * SQ + qb * 128
            blk = am[s0:s0 + 128, t0 + tlo:t0 + 128]  # [s, t]
            pmask[tlo:128, col0:col0 + 128] = blk.T.astype(np.float32)
        in_maps.append({
            "xT": xTc.astype(np.float16),
            "wq": wq.astype(np.float16),
            "wk": wk.astype(np.float16),
            "wv": wv.astype(np.float16),
            "wout": wout.astype(np.float16),
            "pmask": pmask.astype(np.float16),
        })
    return in_maps


_CACHED_NC = None


def kernel(x, allowed_mask, w_qkv, w_out):
    global _CACHED_NC
    if _CACHED_NC is None:
        _CACHED_NC = build()
    in_maps = make_in_maps(x, allowed_mask, w_qkv, w_out)
    res = run_bass_kernel_spmd(_CACHED_NC, in_maps, list(range(N_CORES)))
    out = np.concatenate([res.results[c]["out"] for c in range(N_CORES)], axis=0)
    return out.reshape(B, S, E)
